# revision 11
# baseline (speedup 1.0000x reference)
"""Trainium2 distributed causal attention kernel (8 NeuronCores).

Problem: x[4,2048,1024] -> qkv proj -> 16-head causal attention -> out proj.

Sharding (uniform SPMD graph on all 8 cores):
  core c = (batch b = c//2, head-group g = c%2 of 8 heads).
  Each core: projects q/k/v for its 8 heads over the full 2048 tokens of its
  batch, runs causal flash-style attention (no max subtraction -- scores are
  O(1) for this input distribution), computes the partial output projection
  with its 512 inner dims of w_out, adds b_out/2, then a pairwise
  ReduceScatter(add, bf16) over {2b, 2b+1} yields final output token-stripes.
  Host reassembles stripes. No other collectives.

v2 (all-bf16, stall fixes over the f32r baseline):
  - x / w_qkv / w_out / b_out are converted to bf16 on the host; every matmul
    runs at the bf16 streaming rate.  Weight tiles are persistent in SBUF and
    DMA-ordered so the first projection group's operands land first.
  - diagonal-chunk key blocks only compute the un-masked column range
    (col0 = v*128 trimming on QK matmul, exp, and PV matmul); the pt zero
    memsets disappear.
  - out-proj bias is added by the DVE during psum evacuation against a
    DMA-broadcast [128,1024] bias tile instead of a ones-row matmul.
  - softmax epilogue is per-head-pair: reciprocal of the denominator row is
    taken straight out of the PV psum and its [64,512] broadcast DMA issues
    while the next head-pair is still computing; the chunk epilogue is just
    8 DVE multiplies.
  - chunks are processed in descending cost order (3,2,1,0) and the final
    chunk's out-proj ReduceScatters in four 128-token pieces (bf16), with the
    RS-dependent output stores issued on the gpsimd queue so they cannot
    head-of-line-block the sync DMA queue.
"""

import sys

sys.path.insert(0, "/opt/trn_rl_repo")

import numpy as np

B, N, DM = 4, 2048, 1024
H, DH = 16, 64
HG = 8  # heads per core
LI = HG * DH  # local inner = 512
NCORES = 8
CHUNK = 512  # q-chunk tokens
NCHUNK = N // CHUNK  # 4
KB = 128  # k-block size
VW = DH + 1  # v columns per head incl. ones column
LAST_NRS = 1  # RS split of the final processed chunk (chunk 0); split
# pieces serialize on the CC engine so one piece gives the shortest tail

_GRAPH = None


def _build_graph():
    from concourse import bacc, bass, mybir, tile

    f32 = mybir.dt.float32
    bf16 = mybir.dt.bfloat16
    Exp = mybir.ActivationFunctionType.Exp

    nc = bacc.Bacc("TRN2", target_bir_lowering=False, debug=False)

    xT_d = nc.dram_tensor("xT", [DM, N], bf16, kind="ExternalInput")
    wq_d = nc.dram_tensor("wq", [DM, LI], bf16, kind="ExternalInput")
    wk_d = nc.dram_tensor("wk", [DM, LI], bf16, kind="ExternalInput")
    wv_d = nc.dram_tensor("wv", [DM, LI], bf16, kind="ExternalInput")
    wo_d = nc.dram_tensor("wo", [LI, DM], bf16, kind="ExternalInput")
    hb_d = nc.dram_tensor("hb", [1, DM], bf16, kind="ExternalInput")
    mask_d = nc.dram_tensor("mask", [KB, KB], bf16, kind="ExternalInput")
    out_d = nc.dram_tensor("out", [N // 2, DM], bf16, kind="ExternalOutput")

    RG = [[0, 1], [2, 3], [4, 5], [6, 7]]

    with tile.TileContext(nc) as tc:
        with (
            tc.tile_pool(name="persist", bufs=1) as pers,
            tc.tile_pool(name="xpool", bufs=1) as xpool,
            tc.tile_pool(name="work", bufs=4) as work,
            tc.tile_pool(name="mmps", bufs=2, space="PSUM") as mmps,
            tc.tile_pool(name="simps", bufs=4, space="PSUM") as simps,
            tc.tile_pool(name="pvps", bufs=2, space="PSUM") as pvps,
            tc.tile_pool(name="dram", bufs=2, space="DRAM") as dram,
        ):
            # ---- persistent weights / constants; DMA order matters: the
            # first kq-projection group needs wk + xT token-chunk 0 first ----
            wkt = [pers.tile([128, LI], bf16, tag=f"wk{d}", name=f"wk{d}") for d in range(8)]
            wqt = [pers.tile([128, LI], bf16, tag=f"wq{d}", name=f"wq{d}") for d in range(8)]
            wvt = [pers.tile([128, LI], bf16, tag=f"wv{d}", name=f"wv{d}") for d in range(8)]
            xTc = [[None] * 4 for _ in range(8)]

            def load_x(d, cc):
                t = xpool.tile([128, CHUNK], bf16, tag=f"x{d}_{cc}", name=f"x{d}_{cc}")
                nc.sync.dma_start(
                    out=t[:, :],
                    in_=xT_d[d * 128 : (d + 1) * 128, cc * 512 : (cc + 1) * 512],
                )
                xTc[d][cc] = t

            # d-interleaved so matmul d of the first psum group starts as
            # soon as ITS operands land, not after the whole phase's loads
            for d in range(8):
                nc.sync.dma_start(out=wkt[d][:, :], in_=wk_d[d * 128 : (d + 1) * 128, :])
                load_x(d, 0)
            for d in range(8):
                nc.sync.dma_start(out=wqt[d][:, :], in_=wq_d[d * 128 : (d + 1) * 128, :])
                load_x(d, 1)
            for d in range(8):
                nc.sync.dma_start(out=wvt[d][:, :], in_=wv_d[d * 128 : (d + 1) * 128, :])
                load_x(d, 2)
            for d in range(8):
                load_x(d, 3)

            mask_sb = pers.tile([KB, KB], bf16, tag="mask")
            nc.sync.dma_start(out=mask_sb[:, :], in_=mask_d[:, :])

            wo_bf = []
            for it in range(4):
                wob = pers.tile([128, DM], bf16, tag=f"wo{it}")
                nc.sync.dma_start(out=wob[:, :], in_=wo_d[it * 128 : (it + 1) * 128, :])
                wo_bf.append(wob)

            hb_sb = pers.tile([1, DM], bf16, tag="hb")
            nc.sync.dma_start(out=hb_sb[:, :], in_=hb_d[:, :])
            hbb = pers.tile([128, DM], bf16, tag="hbb")
            hrow = hb_sb[0:1, :]
            hsrc = bass.AP(
                tensor=hrow.tensor,
                offset=hrow.offset,
                ap=[[DM, 1], [0, 128], [1, DM]],
            )
            nc.sync.dma_start(out=hbb[:, :], in_=hsrc)

            # ---- phase 1: projections (all bf16) ----
            kT = [pers.tile([128, N], bf16, tag=f"kT{i}", name=f"kT{i}") for i in range(4)]
            qT = [pers.tile([128, N], bf16, tag=f"qT{i}", name=f"qT{i}") for i in range(4)]

            for wt, dst in ((wkt, kT), (wqt, qT)):
                for tt in range(4):
                    for it in range(4):
                        ps = mmps.tile([128, 512], f32, tag="mm")
                        for d in range(8):
                            nc.tensor.matmul(
                                ps[:, :],
                                lhsT=wt[d][:, it * 128 : (it + 1) * 128],
                                rhs=xTc[d][tt][:, :],
                                start=(d == 0),
                                stop=(d == 7),
                            )
                        nc.vector.tensor_copy(
                            dst[it][:, tt * 512 : (tt + 1) * 512], ps[:, :]
                        )

            v_aug = [pers.tile([128, HG * VW], bf16, tag=f"va{t}", name=f"va{t}") for t in range(16)]
            for tt in range(16):
                va3 = v_aug[tt].rearrange("p (h c) -> p h c", h=HG)
                nc.vector.memset(va3[:, :, DH : DH + 1], 1.0)
                ps = mmps.tile([128, 512], f32, tag="mm")
                for d in range(8):
                    nc.tensor.matmul(
                        ps[:, :],
                        lhsT=xTc[d][tt // 4][:, (tt % 4) * 128 : (tt % 4 + 1) * 128],
                        rhs=wvt[d][:, :],
                        start=(d == 0),
                        stop=(d == 7),
                    )
                nc.vector.tensor_copy(
                    va3[:, :, 0:DH], ps.rearrange("p (h c) -> p h c", h=HG)
                )

            # ---- phases 2+3: attention + out-proj + RS, chunk-pipelined,
            # descending chunk order so the smallest chunk's out-proj is the
            # kernel tail ----
            chunk_state = {}

            def attention_chunk(c):
                nk = 4 * (c + 1)
                vals = [None] * 8
                rbs = [None] * 8
                aos = [
                    work.tile([128, CHUNK], bf16, tag=f"ao{i}", name=f"ao{i}", bufs=2)
                    for i in range(4)
                ]

                def aos_mul(hp):
                    for e in range(2):
                        h = 2 * hp + e
                        nc.vector.tensor_mul(
                            aos[h // 2][64 * (h % 2) : 64 * (h % 2) + 64, :],
                            vals[h][:, :],
                            rbs[h][:, :],
                        )

                for hp in range(4):
                    pvs = [
                        pvps.tile([VW, CHUNK], f32, tag="pv", name="pv")
                        for _ in range(2)
                    ]
                    sims_of = {}

                    def col0_of(jb):
                        v = jb - (nk - 4)
                        return max(0, v) * KB, v

                    def qk_step(jb):
                        col0, _ = col0_of(jb)
                        sims = [
                            simps.tile([128, CHUNK], f32, tag="sim", name="sim")
                            for _ in range(2)
                        ]
                        for e in range(2):
                            nc.tensor.matmul(
                                sims[e][:, col0:CHUNK],
                                lhsT=kT[hp][
                                    64 * e : 64 * e + 64, jb * KB : (jb + 1) * KB
                                ],
                                rhs=qT[hp][
                                    64 * e : 64 * e + 64,
                                    c * CHUNK + col0 : (c + 1) * CHUNK,
                                ],
                                start=True,
                                stop=True,
                            )
                        sims_of[jb] = sims

                    def pv_step(jb):
                        sims = sims_of.pop(jb)
                        col0, v = col0_of(jb)
                        for e in range(2):
                            h = 2 * hp + e
                            pt = work.tile([128, CHUNK], bf16, tag=f"pt{e}", bufs=3)
                            nc.scalar.activation(
                                pt[:, col0:CHUNK],
                                sims[e][:, col0:CHUNK],
                                Exp,
                                scale=float(DH**-0.5),
                            )
                            if v >= 0:
                                nc.vector.tensor_mul(
                                    pt[:, col0 : col0 + KB],
                                    pt[:, col0 : col0 + KB],
                                    mask_sb[:, :],
                                )
                            nc.tensor.matmul(
                                pvs[e][:, col0:CHUNK],
                                lhsT=v_aug[jb][:, h * VW : (h + 1) * VW],
                                rhs=pt[:, col0:CHUNK],
                                start=(jb == 0),
                                stop=(jb == nk - 1),
                            )

                    # 2-deep software pipeline: the exp of step jb has two
                    # qk steps of PE work to hide under before pv needs it
                    qk_step(0)
                    qk_step(1)
                    for jb in range(nk - 2):
                        qk_step(jb + 2)
                        pv_step(jb)
                    pv_step(nk - 2)
                    pv_step(nk - 1)

                    # psum release first (cheap copies clear the WAR hazard
                    # on the pv banks), then the reciprocal/broadcast chain
                    # off the critical path, then the previous head-pair's
                    # aos multiplies (their broadcasts are long arrived).
                    dcp = []
                    for e in range(2):
                        h = 2 * hp + e
                        dc = work.tile([1, CHUNK], f32, tag=f"dcp{e}", bufs=2)
                        nc.vector.tensor_copy(dc[:, :], pvs[e][DH : DH + 1, :])
                        dcp.append(dc)
                        t = work.tile(
                            [DH, CHUNK], bf16, tag=f"pvsb{h}", bufs=2, name=f"pvsb{h}"
                        )
                        nc.vector.tensor_copy(t[:, :], pvs[e][0:DH, :])
                        vals[h] = t
                    for e in range(2):
                        h = 2 * hp + e
                        rc1 = work.tile([1, CHUNK], f32, tag=f"rc{e}", bufs=2)
                        nc.vector.reciprocal_approx_fast(rc1[:, :], dcp[e][:, :])
                        rcb1 = work.tile([1, CHUNK], bf16, tag=f"rcb{e}", bufs=2)
                        nc.vector.tensor_copy(rcb1[:, :], rc1[:, :])
                        rb = work.tile([DH, CHUNK], bf16, tag=f"rb{h}", bufs=2)
                        rrow = rcb1[0:1, :]
                        rsrc = bass.AP(
                            tensor=rrow.tensor,
                            offset=rrow.offset,
                            ap=[[CHUNK, 1], [0, DH], [1, CHUNK]],
                        )
                        nc.sync.dma_start(out=rb[:, :], in_=rsrc)
                        rbs[h] = rb
                    if hp >= 1:
                        aos_mul(hp - 1)
                aos_mul(3)
                chunk_state[c] = aos

            def outproj_chunk(c, n_rs=1):
                aos = chunk_state.pop(c)
                pd = dram.tile([CHUNK, DM], bf16, tag="pd")
                ts_per_rs = 4 // n_rs
                for rs_i in range(n_rs):
                    for ts in range(rs_i * ts_per_rs, (rs_i + 1) * ts_per_rs):
                        for ct in range(2):
                            po = mmps.tile([128, 512], f32, tag="mm")
                            for it in range(4):
                                nc.tensor.matmul(
                                    po[:, :],
                                    lhsT=aos[it][:, ts * 128 : (ts + 1) * 128],
                                    rhs=wo_bf[it][:, ct * 512 : (ct + 1) * 512],
                                    start=(it == 0),
                                    stop=(it == 3),
                                )
                            ob = work.tile([128, 512], bf16, tag="ob", name="ob", bufs=2)
                            nc.vector.tensor_add(
                                ob[:, :], po[:, :], hbb[:, ct * 512 : (ct + 1) * 512]
                            )
                            nc.sync.dma_start(
                                out=pd[
                                    ts * 128 : (ts + 1) * 128,
                                    ct * 512 : (ct + 1) * 512,
                                ],
                                in_=ob[:, :],
                            )
                    rows = CHUNK // n_rs
                    rs = dram.tile(
                        [rows // 2, DM],
                        bf16,
                        tag="rs",
                        name="rs",
                        padded_shape=[CHUNK // 2, DM],
                    )
                    nc.gpsimd.collective_compute(
                        "ReduceScatter",
                        mybir.AluOpType.add,
                        replica_groups=RG,
                        ins=[pd[rs_i * rows : (rs_i + 1) * rows, :].opt()],
                        outs=[rs[:, :].opt()],
                    )
                    out_r0 = c * 256 + rs_i * (rows // 2)
                    nc.gpsimd.dma_start(
                        out=out_d[out_r0 : out_r0 + rows // 2, :], in_=rs[:, :]
                    )

            # schedule: aos multiplies happen inside attention_chunk (one
            # head-pair deferred), so out-proj can directly follow its own
            # chunk and every RS except the last hides under the next
            # chunk's attention matmuls.
            for c in (3, 2, 1, 0):
                attention_chunk(c)
                outproj_chunk(c, n_rs=LAST_NRS if c == 0 else 1)

    nc.finalize()
    return nc


def _get_graph():
    global _GRAPH
    if _GRAPH is None:
        _GRAPH = _build_graph()
    return _GRAPH


def _build_masks():
    # [j, ti] = 1 where ti >= j: token ti attends key j within the diagonal block
    return np.ascontiguousarray(np.triu(np.ones((KB, KB), np.float32)))


def _make_in_maps(x, w_qkv, w_out, b_out):
    import ml_dtypes

    bf = ml_dtypes.bfloat16
    x = np.asarray(x, np.float32)
    w_qkv = np.asarray(w_qkv, np.float32).astype(bf)
    w_out = np.asarray(w_out, np.float32).astype(bf)
    b_out = np.asarray(b_out, np.float32)

    xT = [np.ascontiguousarray(x[b].T).astype(bf) for b in range(B)]
    masks = _build_masks().astype(bf)
    hb = np.ascontiguousarray((0.5 * b_out).reshape(1, DM)).astype(bf)
    in_maps = []
    for c in range(NCORES):
        b, g = c // 2, c % 2
        in_maps.append(
            {
                "xT": xT[b],
                "wq": np.ascontiguousarray(w_qkv[:, LI * g : LI * (g + 1)]),
                "wk": np.ascontiguousarray(w_qkv[:, DM + LI * g : DM + LI * (g + 1)]),
                "wv": np.ascontiguousarray(
                    w_qkv[:, 2 * DM + LI * g : 2 * DM + LI * (g + 1)]
                ),
                "wo": np.ascontiguousarray(w_out[LI * g : LI * (g + 1), :]),
                "hb": hb,
                "mask": masks,
            }
        )
    return in_maps


def _assemble(results):
    y = np.empty((B, N, DM), np.float32)
    for c in range(NCORES):
        b, g = c // 2, c % 2
        o = np.asarray(results[c]["out"], np.float32)  # [1024, 1024] token stripes
        for ch in range(NCHUNK):
            n_rs = LAST_NRS if ch == 0 else 1
            rows_per = CHUNK // n_rs
            half = rows_per // 2
            for p in range(n_rs):
                t0 = ch * CHUNK + p * rows_per + g * half
                r0 = ch * 256 + p * half
                y[b, t0 : t0 + half] = o[r0 : r0 + half]
    return y


def _install_ntff_hook_shim():
    """The container's antenv package lacks axon_hooks; synthesize it so
    run_bass_kernel_spmd(trace=True) can NTFF-profile via the injected .so."""
    import types

    if "antenv.axon_hooks" in sys.modules:
        return
    try:
        from trn_agent_boot.trn_boot import _ntff_profile_via_ctypes

        hook = _ntff_profile_via_ctypes("/opt/axon/libaxon_pjrt.so")
    except Exception as e:  # profiling degrades, run still works
        print(f"ntff hook shim unavailable: {e}")
        hook = None
    mod = types.ModuleType("antenv.axon_hooks")
    _state = {"hook": hook}
    mod.set_axon_ntff_profile_hook = lambda h: _state.__setitem__("hook", h)
    mod.get_axon_ntff_profile_hook = lambda: _state["hook"]
    sys.modules["antenv.axon_hooks"] = mod
    import antenv

    antenv.axon_hooks = mod


def _run(in_maps, trace=False):
    from concourse import bass_utils

    if trace:
        _install_ntff_hook_shim()
    nc = _get_graph()
    return bass_utils.run_bass_kernel_spmd(
        nc, in_maps, core_ids=list(range(NCORES)), trace=trace
    )


def kernel(x, w_qkv, w_out, b_out):
    res = _run(_make_in_maps(x, w_qkv, w_out, b_out), trace=False)
    return _assemble(res.results)


def kernel_timed(x, w_qkv, w_out, b_out):
    res = _run(_make_in_maps(x, w_qkv, w_out, b_out), trace=True)
    return _assemble(res.results), res


# revision 12
# speedup vs baseline: 1.0069x; 1.0069x over previous
"""Trainium2 distributed causal attention kernel (8 NeuronCores).

Problem: x[4,2048,1024] -> qkv proj -> 16-head causal attention -> out proj.

Sharding (uniform SPMD graph on all 8 cores):
  core c = (batch b = c//2, head-group g = c%2 of 8 heads).
  Each core: projects q/k/v for its 8 heads over the full 2048 tokens of its
  batch, runs causal flash-style attention (no max subtraction -- scores are
  O(1) for this input distribution), computes the partial output projection
  with its 512 inner dims of w_out, adds b_out/2, then a pairwise
  ReduceScatter(add, bf16) over {2b, 2b+1} yields final output token-stripes.
  Host reassembles stripes. No other collectives.

v2 (all-bf16, stall fixes over the f32r baseline):
  - x / w_qkv / w_out / b_out are converted to bf16 on the host; every matmul
    runs at the bf16 streaming rate.  Weight tiles are persistent in SBUF and
    DMA-ordered so the first projection group's operands land first.
  - diagonal-chunk key blocks only compute the un-masked column range
    (col0 = v*128 trimming on QK matmul, exp, and PV matmul); the pt zero
    memsets disappear.
  - out-proj bias is added by the DVE during psum evacuation against a
    DMA-broadcast [128,1024] bias tile instead of a ones-row matmul.
  - softmax epilogue is per-head-pair: reciprocal of the denominator row is
    taken straight out of the PV psum and its [64,512] broadcast DMA issues
    while the next head-pair is still computing; the chunk epilogue is just
    8 DVE multiplies.
  - chunks are processed in descending cost order (3,2,1,0) and the final
    chunk's out-proj ReduceScatters in four 128-token pieces (bf16), with the
    RS-dependent output stores issued on the gpsimd queue so they cannot
    head-of-line-block the sync DMA queue.
"""

import sys

sys.path.insert(0, "/opt/trn_rl_repo")

import numpy as np

B, N, DM = 4, 2048, 1024
H, DH = 16, 64
HG = 8  # heads per core
LI = HG * DH  # local inner = 512
NCORES = 8
CHUNK = 512  # q-chunk tokens
NCHUNK = N // CHUNK  # 4
KB = 128  # k-block size
VW = DH + 1  # v columns per head incl. ones column
LAST_NRS = 1  # RS split of the final processed chunk (chunk 0); split
# pieces serialize on the CC engine so one piece gives the shortest tail

_GRAPH = None


def _build_graph():
    from concourse import bacc, bass, mybir, tile

    f32 = mybir.dt.float32
    bf16 = mybir.dt.bfloat16
    Exp = mybir.ActivationFunctionType.Exp

    nc = bacc.Bacc("TRN2", target_bir_lowering=False, debug=False)

    xT_d = nc.dram_tensor("xT", [DM, N], bf16, kind="ExternalInput")
    wq_d = nc.dram_tensor("wq", [DM, LI], bf16, kind="ExternalInput")
    wk_d = nc.dram_tensor("wk", [DM, LI], bf16, kind="ExternalInput")
    wv_d = nc.dram_tensor("wv", [DM, LI], bf16, kind="ExternalInput")
    wo_d = nc.dram_tensor("wo", [LI, DM], bf16, kind="ExternalInput")
    hb_d = nc.dram_tensor("hb", [1, DM], bf16, kind="ExternalInput")
    mask_d = nc.dram_tensor("mask", [KB, KB], bf16, kind="ExternalInput")
    out_d = nc.dram_tensor("out", [N // 2, DM], bf16, kind="ExternalOutput")

    RG = [[0, 1], [2, 3], [4, 5], [6, 7]]

    with tile.TileContext(nc) as tc:
        with (
            tc.tile_pool(name="persist", bufs=1) as pers,
            tc.tile_pool(name="xpool", bufs=1) as xpool,
            tc.tile_pool(name="work", bufs=4) as work,
            tc.tile_pool(name="mmps", bufs=2, space="PSUM") as mmps,
            tc.tile_pool(name="simps", bufs=4, space="PSUM") as simps,
            tc.tile_pool(name="pvps", bufs=2, space="PSUM") as pvps,
            tc.tile_pool(name="dram", bufs=2, space="DRAM") as dram,
        ):
            # ---- persistent weights / constants; DMA order matters: the
            # first kq-projection group needs wk + xT token-chunk 0 first ----
            wkt = [pers.tile([128, LI], bf16, tag=f"wk{d}", name=f"wk{d}") for d in range(8)]
            wqt = [pers.tile([128, LI], bf16, tag=f"wq{d}", name=f"wq{d}") for d in range(8)]
            wvt = [pers.tile([128, LI], bf16, tag=f"wv{d}", name=f"wv{d}") for d in range(8)]
            xTc = [[None] * 4 for _ in range(8)]

            def load_x(d, cc):
                t = xpool.tile([128, CHUNK], bf16, tag=f"x{d}_{cc}", name=f"x{d}_{cc}")
                nc.sync.dma_start(
                    out=t[:, :],
                    in_=xT_d[d * 128 : (d + 1) * 128, cc * 512 : (cc + 1) * 512],
                )
                xTc[d][cc] = t

            # d-interleaved so matmul d of the first psum group starts as
            # soon as ITS operands land, not after the whole phase's loads
            for d in range(8):
                nc.sync.dma_start(out=wkt[d][:, :], in_=wk_d[d * 128 : (d + 1) * 128, :])
                load_x(d, 0)
            for d in range(8):
                nc.sync.dma_start(out=wqt[d][:, :], in_=wq_d[d * 128 : (d + 1) * 128, :])
                load_x(d, 1)
            for d in range(8):
                nc.sync.dma_start(out=wvt[d][:, :], in_=wv_d[d * 128 : (d + 1) * 128, :])
                load_x(d, 2)
            for d in range(8):
                load_x(d, 3)

            mask_sb = pers.tile([KB, KB], bf16, tag="mask")
            nc.sync.dma_start(out=mask_sb[:, :], in_=mask_d[:, :])

            wo_bf = []
            for it in range(4):
                wob = pers.tile([128, DM], bf16, tag=f"wo{it}")
                nc.sync.dma_start(out=wob[:, :], in_=wo_d[it * 128 : (it + 1) * 128, :])
                wo_bf.append(wob)

            hb_sb = pers.tile([1, DM], bf16, tag="hb")
            nc.sync.dma_start(out=hb_sb[:, :], in_=hb_d[:, :])
            hbb = pers.tile([128, DM], bf16, tag="hbb")
            hrow = hb_sb[0:1, :]
            hsrc = bass.AP(
                tensor=hrow.tensor,
                offset=hrow.offset,
                ap=[[DM, 1], [0, 128], [1, DM]],
            )
            nc.sync.dma_start(out=hbb[:, :], in_=hsrc)

            # ---- phase 1: projections (all bf16) ----
            kT = [pers.tile([128, N], bf16, tag=f"kT{i}", name=f"kT{i}") for i in range(4)]
            qT = [pers.tile([128, N], bf16, tag=f"qT{i}", name=f"qT{i}") for i in range(4)]

            for wt, dst in ((wkt, kT), (wqt, qT)):
                for tt in range(4):
                    for it in range(4):
                        ps = mmps.tile([128, 512], f32, tag="mm")
                        for d in range(8):
                            nc.tensor.matmul(
                                ps[:, :],
                                lhsT=wt[d][:, it * 128 : (it + 1) * 128],
                                rhs=xTc[d][tt][:, :],
                                start=(d == 0),
                                stop=(d == 7),
                            )
                        nc.vector.tensor_copy(
                            dst[it][:, tt * 512 : (tt + 1) * 512], ps[:, :]
                        )

            v_aug = [pers.tile([128, HG * VW], bf16, tag=f"va{t}", name=f"va{t}") for t in range(16)]
            for tt in range(16):
                va3 = v_aug[tt].rearrange("p (h c) -> p h c", h=HG)
                nc.vector.memset(va3[:, :, DH : DH + 1], 1.0)
                ps = mmps.tile([128, 512], f32, tag="mm")
                for d in range(8):
                    nc.tensor.matmul(
                        ps[:, :],
                        lhsT=xTc[d][tt // 4][:, (tt % 4) * 128 : (tt % 4 + 1) * 128],
                        rhs=wvt[d][:, :],
                        start=(d == 0),
                        stop=(d == 7),
                    )
                nc.vector.tensor_copy(
                    va3[:, :, 0:DH], ps.rearrange("p (h c) -> p h c", h=HG)
                )

            # ---- phases 2+3: attention + out-proj + RS, chunk-pipelined,
            # descending chunk order so the smallest chunk's out-proj is the
            # kernel tail ----
            chunk_state = {}

            def attention_chunk(c):
                nk = 4 * (c + 1)
                vals = [None] * 8
                rbs = [None] * 8
                aos = [
                    work.tile([128, CHUNK], bf16, tag=f"ao{i}", name=f"ao{i}", bufs=2)
                    for i in range(4)
                ]

                def aos_mul(hp):
                    for e in range(2):
                        h = 2 * hp + e
                        nc.vector.tensor_mul(
                            aos[h // 2][64 * (h % 2) : 64 * (h % 2) + 64, :],
                            vals[h][:, :],
                            rbs[h][:, :],
                        )

                for hp in range(4):
                    pvs = [
                        pvps.tile([VW, CHUNK], f32, tag="pv", name="pv")
                        for _ in range(2)
                    ]
                    sims_of = {}

                    def col0_of(jb):
                        v = jb - (nk - 4)
                        return max(0, v) * KB, v

                    def qk_step(jb):
                        col0, _ = col0_of(jb)
                        sims = [
                            simps.tile([128, CHUNK], f32, tag="sim", name="sim")
                            for _ in range(2)
                        ]
                        for e in range(2):
                            nc.tensor.matmul(
                                sims[e][:, col0:CHUNK],
                                lhsT=kT[hp][
                                    64 * e : 64 * e + 64, jb * KB : (jb + 1) * KB
                                ],
                                rhs=qT[hp][
                                    64 * e : 64 * e + 64,
                                    c * CHUNK + col0 : (c + 1) * CHUNK,
                                ],
                                start=True,
                                stop=True,
                            )
                        sims_of[jb] = sims

                    def pv_step(jb):
                        sims = sims_of.pop(jb)
                        col0, v = col0_of(jb)
                        for e in range(2):
                            h = 2 * hp + e
                            pt = work.tile([128, CHUNK], bf16, tag=f"pt{e}", bufs=3)
                            nc.scalar.activation(
                                pt[:, col0:CHUNK],
                                sims[e][:, col0:CHUNK],
                                Exp,
                                scale=float(DH**-0.5),
                            )
                            if v >= 0:
                                nc.vector.tensor_mul(
                                    pt[:, col0 : col0 + KB],
                                    pt[:, col0 : col0 + KB],
                                    mask_sb[:, :],
                                )
                            nc.tensor.matmul(
                                pvs[e][:, col0:CHUNK],
                                lhsT=v_aug[jb][:, h * VW : (h + 1) * VW],
                                rhs=pt[:, col0:CHUNK],
                                start=(jb == 0),
                                stop=(jb == nk - 1),
                            )

                    # 1-deep software pipeline; with 4 sim psum banks the
                    # qk of step jb+1 reuses buffers two steps back, so it
                    # never waits on the exp of step jb (deeper pipelining
                    # consumes that buffer slack and makes qk gate on exp)
                    qk_step(0)
                    for jb in range(1, nk):
                        qk_step(jb)
                        pv_step(jb - 1)
                    pv_step(nk - 1)

                    # psum release first (cheap copies clear the WAR hazard
                    # on the pv banks), then the reciprocal/broadcast chain
                    # off the critical path, then the previous head-pair's
                    # aos multiplies (their broadcasts are long arrived).
                    dcp = []
                    for e in range(2):
                        h = 2 * hp + e
                        dc = work.tile([1, CHUNK], f32, tag=f"dcp{e}", bufs=2)
                        nc.vector.tensor_copy(dc[:, :], pvs[e][DH : DH + 1, :])
                        dcp.append(dc)
                        t = work.tile(
                            [DH, CHUNK], bf16, tag=f"pvsb{h}", bufs=2, name=f"pvsb{h}"
                        )
                        nc.vector.tensor_copy(t[:, :], pvs[e][0:DH, :])
                        vals[h] = t
                    for e in range(2):
                        h = 2 * hp + e
                        rc1 = work.tile([1, CHUNK], f32, tag=f"rc{e}", bufs=2)
                        nc.vector.reciprocal_approx_fast(rc1[:, :], dcp[e][:, :])
                        rcb1 = work.tile([1, CHUNK], bf16, tag=f"rcb{e}", bufs=2)
                        nc.vector.tensor_copy(rcb1[:, :], rc1[:, :])
                        rb = work.tile([DH, CHUNK], bf16, tag=f"rb{h}", bufs=2)
                        rrow = rcb1[0:1, :]
                        rsrc = bass.AP(
                            tensor=rrow.tensor,
                            offset=rrow.offset,
                            ap=[[CHUNK, 1], [0, DH], [1, CHUNK]],
                        )
                        nc.sync.dma_start(out=rb[:, :], in_=rsrc)
                        rbs[h] = rb
                    if hp >= 1:
                        aos_mul(hp - 1)
                aos_mul(3)
                chunk_state[c] = aos

            def outproj_chunk(c, n_rs=1):
                aos = chunk_state.pop(c)
                pd = dram.tile([CHUNK, DM], bf16, tag="pd")
                ts_per_rs = 4 // n_rs
                for rs_i in range(n_rs):
                    for ts in range(rs_i * ts_per_rs, (rs_i + 1) * ts_per_rs):
                        for ct in range(2):
                            po = mmps.tile([128, 512], f32, tag="mm")
                            for it in range(4):
                                nc.tensor.matmul(
                                    po[:, :],
                                    lhsT=aos[it][:, ts * 128 : (ts + 1) * 128],
                                    rhs=wo_bf[it][:, ct * 512 : (ct + 1) * 512],
                                    start=(it == 0),
                                    stop=(it == 3),
                                )
                            ob = work.tile([128, 512], bf16, tag="ob", name="ob", bufs=2)
                            nc.vector.tensor_add(
                                ob[:, :], po[:, :], hbb[:, ct * 512 : (ct + 1) * 512]
                            )
                            nc.sync.dma_start(
                                out=pd[
                                    ts * 128 : (ts + 1) * 128,
                                    ct * 512 : (ct + 1) * 512,
                                ],
                                in_=ob[:, :],
                            )
                    rows = CHUNK // n_rs
                    rs = dram.tile(
                        [rows // 2, DM],
                        bf16,
                        tag="rs",
                        name="rs",
                        padded_shape=[CHUNK // 2, DM],
                    )
                    nc.gpsimd.collective_compute(
                        "ReduceScatter",
                        mybir.AluOpType.add,
                        replica_groups=RG,
                        ins=[pd[rs_i * rows : (rs_i + 1) * rows, :].opt()],
                        outs=[rs[:, :].opt()],
                    )
                    out_r0 = c * 256 + rs_i * (rows // 2)
                    nc.gpsimd.dma_start(
                        out=out_d[out_r0 : out_r0 + rows // 2, :], in_=rs[:, :]
                    )

            # schedule: aos multiplies happen inside attention_chunk (one
            # head-pair deferred), so out-proj can directly follow its own
            # chunk and every RS except the last hides under the next
            # chunk's attention matmuls.
            for c in (3, 2, 1, 0):
                attention_chunk(c)
                outproj_chunk(c, n_rs=LAST_NRS if c == 0 else 1)

    nc.finalize()
    return nc


def _get_graph():
    global _GRAPH
    if _GRAPH is None:
        _GRAPH = _build_graph()
    return _GRAPH


def _build_masks():
    # [j, ti] = 1 where ti >= j: token ti attends key j within the diagonal block
    return np.ascontiguousarray(np.triu(np.ones((KB, KB), np.float32)))


def _make_in_maps(x, w_qkv, w_out, b_out):
    import ml_dtypes

    bf = ml_dtypes.bfloat16
    x = np.asarray(x, np.float32)
    w_qkv = np.asarray(w_qkv, np.float32).astype(bf)
    w_out = np.asarray(w_out, np.float32).astype(bf)
    b_out = np.asarray(b_out, np.float32)

    xT = [np.ascontiguousarray(x[b].T).astype(bf) for b in range(B)]
    masks = _build_masks().astype(bf)
    hb = np.ascontiguousarray((0.5 * b_out).reshape(1, DM)).astype(bf)
    in_maps = []
    for c in range(NCORES):
        b, g = c // 2, c % 2
        in_maps.append(
            {
                "xT": xT[b],
                "wq": np.ascontiguousarray(w_qkv[:, LI * g : LI * (g + 1)]),
                "wk": np.ascontiguousarray(w_qkv[:, DM + LI * g : DM + LI * (g + 1)]),
                "wv": np.ascontiguousarray(
                    w_qkv[:, 2 * DM + LI * g : 2 * DM + LI * (g + 1)]
                ),
                "wo": np.ascontiguousarray(w_out[LI * g : LI * (g + 1), :]),
                "hb": hb,
                "mask": masks,
            }
        )
    return in_maps


def _assemble(results):
    y = np.empty((B, N, DM), np.float32)
    for c in range(NCORES):
        b, g = c // 2, c % 2
        o = np.asarray(results[c]["out"], np.float32)  # [1024, 1024] token stripes
        for ch in range(NCHUNK):
            n_rs = LAST_NRS if ch == 0 else 1
            rows_per = CHUNK // n_rs
            half = rows_per // 2
            for p in range(n_rs):
                t0 = ch * CHUNK + p * rows_per + g * half
                r0 = ch * 256 + p * half
                y[b, t0 : t0 + half] = o[r0 : r0 + half]
    return y


def _install_ntff_hook_shim():
    """The container's antenv package lacks axon_hooks; synthesize it so
    run_bass_kernel_spmd(trace=True) can NTFF-profile via the injected .so."""
    import types

    if "antenv.axon_hooks" in sys.modules:
        return
    try:
        from trn_agent_boot.trn_boot import _ntff_profile_via_ctypes

        hook = _ntff_profile_via_ctypes("/opt/axon/libaxon_pjrt.so")
    except Exception as e:  # profiling degrades, run still works
        print(f"ntff hook shim unavailable: {e}")
        hook = None
    mod = types.ModuleType("antenv.axon_hooks")
    _state = {"hook": hook}
    mod.set_axon_ntff_profile_hook = lambda h: _state.__setitem__("hook", h)
    mod.get_axon_ntff_profile_hook = lambda: _state["hook"]
    sys.modules["antenv.axon_hooks"] = mod
    import antenv

    antenv.axon_hooks = mod


def _run(in_maps, trace=False):
    from concourse import bass_utils

    if trace:
        _install_ntff_hook_shim()
    nc = _get_graph()
    return bass_utils.run_bass_kernel_spmd(
        nc, in_maps, core_ids=list(range(NCORES)), trace=trace
    )


def kernel(x, w_qkv, w_out, b_out):
    res = _run(_make_in_maps(x, w_qkv, w_out, b_out), trace=False)
    return _assemble(res.results)


def kernel_timed(x, w_qkv, w_out, b_out):
    res = _run(_make_in_maps(x, w_qkv, w_out, b_out), trace=True)
    return _assemble(res.results), res


# revision 13
# speedup vs baseline: 1.1596x; 1.1516x over previous
"""Trainium2 distributed causal attention kernel (8 NeuronCores).

Problem: x[4,2048,1024] -> qkv proj -> 16-head causal attention -> out proj.

Sharding (uniform SPMD graph on all 8 cores):
  core c = (batch b = c//2, head-group g = c%2 of 8 heads).
  Each core: projects q/k/v for its 8 heads over the full 2048 tokens of its
  batch, runs causal flash-style attention (no max subtraction -- scores are
  O(1) for this input distribution), computes the partial output projection
  with its 512 inner dims of w_out, adds b_out/2, then a pairwise
  ReduceScatter(add, bf16) over {2b, 2b+1} yields final output token-stripes.
  Host reassembles stripes. No other collectives.

v2 (all-bf16, stall fixes over the f32r baseline):
  - x / w_qkv / w_out / b_out are converted to bf16 on the host; every matmul
    runs at the bf16 streaming rate.  Weight tiles are persistent in SBUF and
    DMA-ordered so the first projection group's operands land first.
  - diagonal-chunk key blocks only compute the un-masked column range
    (col0 = v*128 trimming on QK matmul, exp, and PV matmul); the pt zero
    memsets disappear.
  - out-proj bias is added by the DVE during psum evacuation against a
    DMA-broadcast [128,1024] bias tile instead of a ones-row matmul.
  - softmax epilogue is per-head-pair: reciprocal of the denominator row is
    taken straight out of the PV psum and its [64,512] broadcast DMA issues
    while the next head-pair is still computing; the chunk epilogue is just
    8 DVE multiplies.
  - chunks are processed in descending cost order (3,2,1,0) and the final
    chunk's out-proj ReduceScatters in four 128-token pieces (bf16), with the
    RS-dependent output stores issued on the gpsimd queue so they cannot
    head-of-line-block the sync DMA queue.
"""

import sys

sys.path.insert(0, "/opt/trn_rl_repo")

import numpy as np

B, N, DM = 4, 2048, 1024
H, DH = 16, 64
HG = 8  # heads per core
LI = HG * DH  # local inner = 512
NCORES = 8
CHUNK = 512  # q-chunk tokens
NCHUNK = N // CHUNK  # 4
KB = 128  # k-block size
VW = DH + 1  # v columns per head incl. ones column
LAST_NRS = 1  # RS split of the final processed chunk (chunk 0); split
# pieces serialize on the CC engine so one piece gives the shortest tail

_GRAPH = None


def _build_graph():
    from concourse import bacc, bass, mybir, tile

    f32 = mybir.dt.float32
    bf16 = mybir.dt.bfloat16
    Exp = mybir.ActivationFunctionType.Exp

    nc = bacc.Bacc("TRN2", target_bir_lowering=False, debug=False)

    xT_d = nc.dram_tensor("xT", [DM, N], bf16, kind="ExternalInput")
    wq_d = nc.dram_tensor("wq", [DM, LI], bf16, kind="ExternalInput")
    wk_d = nc.dram_tensor("wk", [DM, LI], bf16, kind="ExternalInput")
    wv_d = nc.dram_tensor("wv", [DM, LI], bf16, kind="ExternalInput")
    wo_d = nc.dram_tensor("wo", [LI, DM], bf16, kind="ExternalInput")
    hb_d = nc.dram_tensor("hb", [1, DM], bf16, kind="ExternalInput")
    mask_d = nc.dram_tensor("mask", [KB, KB], bf16, kind="ExternalInput")
    out_d = nc.dram_tensor("out", [N // 2, DM], bf16, kind="ExternalOutput")

    RG = [[0, 1], [2, 3], [4, 5], [6, 7]]

    with tile.TileContext(nc) as tc:
        with (
            tc.tile_pool(name="persist", bufs=1) as pers,
            tc.tile_pool(name="xpool", bufs=1) as xpool,
            tc.tile_pool(name="work", bufs=4) as work,
            tc.tile_pool(name="mmps", bufs=2, space="PSUM") as mmps,
            tc.tile_pool(name="simps", bufs=4, space="PSUM") as simps,
            tc.tile_pool(name="pvps", bufs=2, space="PSUM") as pvps,
            tc.tile_pool(name="dram", bufs=2, space="DRAM") as dram,
        ):
            # ---- persistent weights / constants; DMA order matters: the
            # first kq-projection group needs wk + xT token-chunk 0 first ----
            wkt = [pers.tile([128, LI], bf16, tag=f"wk{d}", name=f"wk{d}") for d in range(8)]
            wqt = [pers.tile([128, LI], bf16, tag=f"wq{d}", name=f"wq{d}") for d in range(8)]
            wvt = [pers.tile([128, LI], bf16, tag=f"wv{d}", name=f"wv{d}") for d in range(8)]
            xTc = [[None] * 4 for _ in range(8)]

            def load_x(d, cc):
                t = xpool.tile([128, CHUNK], bf16, tag=f"x{d}_{cc}", name=f"x{d}_{cc}")
                nc.sync.dma_start(
                    out=t[:, :],
                    in_=xT_d[d * 128 : (d + 1) * 128, cc * 512 : (cc + 1) * 512],
                )
                xTc[d][cc] = t

            # d-interleaved so matmul d of the first psum group starts as
            # soon as ITS operands land, not after the whole phase's loads
            for d in range(8):
                nc.sync.dma_start(out=wkt[d][:, :], in_=wk_d[d * 128 : (d + 1) * 128, :])
                load_x(d, 0)
            for d in range(8):
                nc.sync.dma_start(out=wqt[d][:, :], in_=wq_d[d * 128 : (d + 1) * 128, :])
                load_x(d, 1)
            for d in range(8):
                nc.sync.dma_start(out=wvt[d][:, :], in_=wv_d[d * 128 : (d + 1) * 128, :])
                load_x(d, 2)
            for d in range(8):
                load_x(d, 3)

            mask_sb = pers.tile([KB, KB], bf16, tag="mask")
            nc.sync.dma_start(out=mask_sb[:, :], in_=mask_d[:, :])

            wo_bf = []
            for it in range(4):
                wob = pers.tile([128, DM], bf16, tag=f"wo{it}")
                nc.sync.dma_start(out=wob[:, :], in_=wo_d[it * 128 : (it + 1) * 128, :])
                wo_bf.append(wob)

            hb_sb = pers.tile([1, DM], bf16, tag="hb")
            nc.sync.dma_start(out=hb_sb[:, :], in_=hb_d[:, :])
            hbb = pers.tile([128, DM], bf16, tag="hbb")
            hrow = hb_sb[0:1, :]
            hsrc = bass.AP(
                tensor=hrow.tensor,
                offset=hrow.offset,
                ap=[[DM, 1], [0, 128], [1, DM]],
            )
            nc.sync.dma_start(out=hbb[:, :], in_=hsrc)

            # ---- phase 1: projections (all bf16) ----
            kT = [pers.tile([128, N], bf16, tag=f"kT{i}", name=f"kT{i}") for i in range(4)]
            qT = [pers.tile([128, N], bf16, tag=f"qT{i}", name=f"qT{i}") for i in range(4)]

            for wt, dst in ((wkt, kT), (wqt, qT)):
                for tt in range(4):
                    for it in range(4):
                        ps = mmps.tile([128, 512], f32, tag="mm")
                        for d in range(8):
                            nc.tensor.matmul(
                                ps[:, :],
                                lhsT=wt[d][:, it * 128 : (it + 1) * 128],
                                rhs=xTc[d][tt][:, :],
                                start=(d == 0),
                                stop=(d == 7),
                            )
                        nc.vector.tensor_copy(
                            dst[it][:, tt * 512 : (tt + 1) * 512], ps[:, :]
                        )

            v_aug = [pers.tile([128, HG * VW], bf16, tag=f"va{t}", name=f"va{t}") for t in range(16)]
            for tt in range(16):
                va3 = v_aug[tt].rearrange("p (h c) -> p h c", h=HG)
                nc.vector.memset(va3[:, :, DH : DH + 1], 1.0)
                ps = mmps.tile([128, 512], f32, tag="mm")
                for d in range(8):
                    nc.tensor.matmul(
                        ps[:, :],
                        lhsT=xTc[d][tt // 4][:, (tt % 4) * 128 : (tt % 4 + 1) * 128],
                        rhs=wvt[d][:, :],
                        start=(d == 0),
                        stop=(d == 7),
                    )
                nc.vector.tensor_copy(
                    va3[:, :, 0:DH], ps.rearrange("p (h c) -> p h c", h=HG)
                )

            # ---- phases 2+3: attention + out-proj + RS, chunk-pipelined,
            # descending chunk order so the smallest chunk's out-proj is the
            # kernel tail ----
            chunk_state = {}

            def attention_chunk(c):
                nk = 4 * (c + 1)
                vals = [None] * 8
                rbs = [None] * 8
                aos = [
                    work.tile([128, CHUNK], bf16, tag=f"ao{i}", name=f"ao{i}", bufs=2)
                    for i in range(4)
                ]

                def aos_mul(hp):
                    for e in range(2):
                        h = 2 * hp + e
                        nc.vector.tensor_mul(
                            aos[h // 2][64 * (h % 2) : 64 * (h % 2) + 64, :],
                            vals[h][:, :],
                            rbs[h][:, :],
                        )

                for hp in range(4):
                    pvs = [
                        pvps.tile([VW, CHUNK], f32, tag="pv", name="pv")
                        for _ in range(2)
                    ]
                    sims_of = {}

                    def col0_of(jb):
                        v = jb - (nk - 4)
                        return max(0, v) * KB, v

                    def qk_step(jb):
                        col0, _ = col0_of(jb)
                        sims = [
                            simps.tile([128, CHUNK], f32, tag="sim", name="sim")
                            for _ in range(2)
                        ]
                        for e in range(2):
                            nc.tensor.matmul(
                                sims[e][:, col0:CHUNK],
                                lhsT=kT[hp][
                                    64 * e : 64 * e + 64, jb * KB : (jb + 1) * KB
                                ],
                                rhs=qT[hp][
                                    64 * e : 64 * e + 64,
                                    c * CHUNK + col0 : (c + 1) * CHUNK,
                                ],
                                start=True,
                                stop=True,
                            )
                        sims_of[jb] = sims

                    def pv_step(jb):
                        sims = sims_of.pop(jb)
                        col0, v = col0_of(jb)
                        for e in range(2):
                            h = 2 * hp + e
                            pt = work.tile([128, CHUNK], bf16, tag=f"pt{e}", bufs=3)
                            nc.scalar.activation(
                                pt[:, col0:CHUNK],
                                sims[e][:, col0:CHUNK],
                                Exp,
                                scale=float(DH**-0.5),
                            )
                            if v >= 0:
                                nc.vector.tensor_mul(
                                    pt[:, col0 : col0 + KB],
                                    pt[:, col0 : col0 + KB],
                                    mask_sb[:, :],
                                )
                            nc.tensor.matmul(
                                pvs[e][:, col0:CHUNK],
                                lhsT=v_aug[jb][:, h * VW : (h + 1) * VW],
                                rhs=pt[:, col0:CHUNK],
                                start=(jb == 0),
                                stop=(jb == nk - 1),
                            )

                    # 1-deep software pipeline; with 4 sim psum banks the
                    # qk of step jb+1 reuses buffers two steps back, so it
                    # never waits on the exp of step jb (deeper pipelining
                    # consumes that buffer slack and makes qk gate on exp)
                    qk_step(0)
                    for jb in range(1, nk):
                        qk_step(jb)
                        pv_step(jb - 1)
                    pv_step(nk - 1)

                    # psum release first (cheap copies clear the WAR hazard
                    # on the pv banks), then the reciprocal/broadcast chain
                    # off the critical path, then the previous head-pair's
                    # aos multiplies (their broadcasts are long arrived).
                    dcp = []
                    for e in range(2):
                        h = 2 * hp + e
                        dc = work.tile([1, CHUNK], f32, tag=f"dcp{e}", bufs=2)
                        nc.vector.tensor_copy(dc[:, :], pvs[e][DH : DH + 1, :])
                        dcp.append(dc)
                        t = work.tile(
                            [DH, CHUNK], bf16, tag=f"pvsb{h}", bufs=2, name=f"pvsb{h}"
                        )
                        nc.vector.tensor_copy(t[:, :], pvs[e][0:DH, :])
                        vals[h] = t
                    for e in range(2):
                        h = 2 * hp + e
                        rc1 = work.tile([1, CHUNK], f32, tag=f"rc{e}", bufs=2)
                        nc.vector.reciprocal_approx_fast(rc1[:, :], dcp[e][:, :])
                        rcb1 = work.tile([1, CHUNK], bf16, tag=f"rcb{e}", bufs=2)
                        nc.vector.tensor_copy(rcb1[:, :], rc1[:, :])
                        rb = work.tile([DH, CHUNK], bf16, tag=f"rb{h}", bufs=2)
                        rrow = rcb1[0:1, :]
                        rsrc = bass.AP(
                            tensor=rrow.tensor,
                            offset=rrow.offset,
                            ap=[[CHUNK, 1], [0, DH], [1, CHUNK]],
                        )
                        nc.sync.dma_start(out=rb[:, :], in_=rsrc)
                        rbs[h] = rb
                    if hp >= 1:
                        aos_mul(hp - 1)
                aos_mul(3)
                chunk_state[c] = aos

            def outproj_chunk(c, n_rs=1):
                aos = chunk_state.pop(c)
                pd = dram.tile([CHUNK, DM], bf16, tag="pd")
                ts_per_rs = 4 // n_rs
                for rs_i in range(n_rs):
                    for ts in range(rs_i * ts_per_rs, (rs_i + 1) * ts_per_rs):
                        for ct in range(2):
                            po = mmps.tile([128, 512], f32, tag="mm")
                            for it in range(4):
                                nc.tensor.matmul(
                                    po[:, :],
                                    lhsT=aos[it][:, ts * 128 : (ts + 1) * 128],
                                    rhs=wo_bf[it][:, ct * 512 : (ct + 1) * 512],
                                    start=(it == 0),
                                    stop=(it == 3),
                                )
                            ob = work.tile([128, 512], bf16, tag="ob", name="ob", bufs=2)
                            nc.vector.tensor_add(
                                ob[:, :], po[:, :], hbb[:, ct * 512 : (ct + 1) * 512]
                            )
                            nc.sync.dma_start(
                                out=pd[
                                    ts * 128 : (ts + 1) * 128,
                                    ct * 512 : (ct + 1) * 512,
                                ],
                                in_=ob[:, :],
                            )
                    rows = CHUNK // n_rs
                    rs = dram.tile(
                        [rows // 2, DM],
                        bf16,
                        tag="rs",
                        name="rs",
                        padded_shape=[CHUNK // 2, DM],
                    )
                    nc.gpsimd.collective_compute(
                        "ReduceScatter",
                        mybir.AluOpType.add,
                        replica_groups=RG,
                        ins=[pd[rs_i * rows : (rs_i + 1) * rows, :].opt()],
                        outs=[rs[:, :].opt()],
                    )
                    out_r0 = c * 256 + rs_i * (rows // 2)
                    nc.gpsimd.dma_start(
                        out=out_d[out_r0 : out_r0 + rows // 2, :], in_=rs[:, :]
                    )

            # schedule: out-proj of chunk c is emitted after the NEXT chunk's
            # attention -- issuing it right after its own chunk queues its
            # DVE evacuations ahead of the next chunk's softmax muls and
            # cascade-stalls the PE.  Chunk 1's out-proj is pulled ahead of
            # att(0) so RS(1) hides under chunk-0 compute and only RS(0) is
            # exposed at the tail.
            attention_chunk(3)
            attention_chunk(2)
            outproj_chunk(3)
            attention_chunk(1)
            outproj_chunk(2)
            outproj_chunk(1)
            attention_chunk(0)
            outproj_chunk(0, n_rs=LAST_NRS)

    nc.finalize()
    return nc


def _get_graph():
    global _GRAPH
    if _GRAPH is None:
        _GRAPH = _build_graph()
    return _GRAPH


def _build_masks():
    # [j, ti] = 1 where ti >= j: token ti attends key j within the diagonal block
    return np.ascontiguousarray(np.triu(np.ones((KB, KB), np.float32)))


def _make_in_maps(x, w_qkv, w_out, b_out):
    import ml_dtypes

    bf = ml_dtypes.bfloat16
    x = np.asarray(x, np.float32)
    w_qkv = np.asarray(w_qkv, np.float32).astype(bf)
    w_out = np.asarray(w_out, np.float32).astype(bf)
    b_out = np.asarray(b_out, np.float32)

    xT = [np.ascontiguousarray(x[b].T).astype(bf) for b in range(B)]
    masks = _build_masks().astype(bf)
    hb = np.ascontiguousarray((0.5 * b_out).reshape(1, DM)).astype(bf)
    in_maps = []
    for c in range(NCORES):
        b, g = c // 2, c % 2
        in_maps.append(
            {
                "xT": xT[b],
                "wq": np.ascontiguousarray(w_qkv[:, LI * g : LI * (g + 1)]),
                "wk": np.ascontiguousarray(w_qkv[:, DM + LI * g : DM + LI * (g + 1)]),
                "wv": np.ascontiguousarray(
                    w_qkv[:, 2 * DM + LI * g : 2 * DM + LI * (g + 1)]
                ),
                "wo": np.ascontiguousarray(w_out[LI * g : LI * (g + 1), :]),
                "hb": hb,
                "mask": masks,
            }
        )
    return in_maps


def _assemble(results):
    y = np.empty((B, N, DM), np.float32)
    for c in range(NCORES):
        b, g = c // 2, c % 2
        o = np.asarray(results[c]["out"], np.float32)  # [1024, 1024] token stripes
        for ch in range(NCHUNK):
            n_rs = LAST_NRS if ch == 0 else 1
            rows_per = CHUNK // n_rs
            half = rows_per // 2
            for p in range(n_rs):
                t0 = ch * CHUNK + p * rows_per + g * half
                r0 = ch * 256 + p * half
                y[b, t0 : t0 + half] = o[r0 : r0 + half]
    return y


def _install_ntff_hook_shim():
    """The container's antenv package lacks axon_hooks; synthesize it so
    run_bass_kernel_spmd(trace=True) can NTFF-profile via the injected .so."""
    import types

    if "antenv.axon_hooks" in sys.modules:
        return
    try:
        from trn_agent_boot.trn_boot import _ntff_profile_via_ctypes

        hook = _ntff_profile_via_ctypes("/opt/axon/libaxon_pjrt.so")
    except Exception as e:  # profiling degrades, run still works
        print(f"ntff hook shim unavailable: {e}")
        hook = None
    mod = types.ModuleType("antenv.axon_hooks")
    _state = {"hook": hook}
    mod.set_axon_ntff_profile_hook = lambda h: _state.__setitem__("hook", h)
    mod.get_axon_ntff_profile_hook = lambda: _state["hook"]
    sys.modules["antenv.axon_hooks"] = mod
    import antenv

    antenv.axon_hooks = mod


def _run(in_maps, trace=False):
    from concourse import bass_utils

    if trace:
        _install_ntff_hook_shim()
    nc = _get_graph()
    return bass_utils.run_bass_kernel_spmd(
        nc, in_maps, core_ids=list(range(NCORES)), trace=trace
    )


def kernel(x, w_qkv, w_out, b_out):
    res = _run(_make_in_maps(x, w_qkv, w_out, b_out), trace=False)
    return _assemble(res.results)


def kernel_timed(x, w_qkv, w_out, b_out):
    res = _run(_make_in_maps(x, w_qkv, w_out, b_out), trace=True)
    return _assemble(res.results), res
